# revision 16
# baseline (speedup 1.0000x reference)
"""Trainium2 Bass kernel for a YOLO-style detection loss.

Sharding: data-parallel over batch — 8 NeuronCores, 4 batches/core.
Per-core final partial sums land in a [128, 16] tile; the host sums the
relevant slices of the 8 tiles and assembles the 4 scalar losses
(this host gather replaces the all-reduce of 4 scalars).

The loss decomposes so that the device only needs:

1. Dense class stream (channels-last [cells, 80] layout, host-prepared):
     cls_dense = sum_cells cnt[cell] * sum_k softplus(x[cell, k])
   Streamed softplus (ACT, in place) + fused multiply-reduce (DVE
   tensor_tensor_reduce) with the per-cell count row broadcast along the
   free (channel) axis.  cnt counts targets assigned to the cell.

2. Dense objectness stream (channel 4 only, [cells] contiguous):
     obj_sp[s] = sum softplus(x)     (= sum of BCE(x, 0))
   via ACT softplus with fused per-partition accumulate.

3. Sparse per-target part: only 6 elements of pred matter per target —
   channels 0..3 (box decode), channel 4 (positive-cell correction:
   BCE(x,1)-BCE(x,0) = -x) and channel 5+cls (one-hot correction, also
   -x).  Host computes flat offsets; one indirect DMA gathers 5-float
   rows from a channels-first-5 copy, a second gathers the class logit.

Normalisations and cross-scale sums are applied on the host from the
partial sums.
"""

import numpy as np

from concourse import bass, bacc, mybir
from concourse import bass_utils
from concourse.tile import TileContext

F32 = mybir.dt.float32
I32 = mybir.dt.int32

NUM_CLASSES = 80
STAL_GAMMA = np.float32(2.0)
BATCH = 32
NCORES = 8
BPC = BATCH // NCORES          # batches per core
CH = 5 + NUM_CLASSES
HW = (80 * 80, 40 * 40, 20 * 20)
WS = (80, 40, 20)
NCELL = BPC * (HW[0] + HW[1] + HW[2])       # 33600 cells per core
COFF = (0, BPC * HW[0], BPC * (HW[0] + HW[1]))  # per-scale cell offsets
# FCLS chunking: cells padded to 4 chunks x 128 partitions x 66 cells
KCELL = 66
NCHUNK = 4
CELLS_PC = 128 * KCELL                      # 8448 cells per chunk
NCELL_PAD = NCHUNK * CELLS_PC               # 33792
FREE = KCELL * NUM_CLASSES                  # 5280 f32 per partition-row
# OBJ stream: per-scale tiles, scale 2 padded to a multiple of 128
OBJ_COLS = (HW[0] * BPC // 128, HW[1] * BPC // 128, 1664 // 128)  # 200,50,13
NOBJ = HW[0] * BPC + HW[1] * BPC + 1664     # 33664 (64 pad cells of -100)
TPAD = 512                                  # per-core target capacity
# meta column layout (groups of 4 target-columns, channel-interleaved)
MC_ADD = 0      # (gx, gy) per target            cols 0..7
MC_MUL = 8      # (1/w, 1/h, 1/w, 1/h) = 1/w x4  cols 8..23
MC_SUB = 24     # (cx, cy, bw, bh)               cols 24..39
MC_SWM = 40     # small_weight/4 * mask          cols 40..43
MC_D0 = 44      # obj dedup flag per scale       cols 44..55
MC_VC = 56      # valid-class flag               cols 56..59
NMETA = 60
# output partial tile column layout
OC_WSP = 0      # NCHUNK cols: weighted class softplus sums
OC_OBJ = 4      # 3 cols: per-scale objectness softplus sums
OC_BOX = 7
OC_POS = 8      # 3 cols
OC_CORR = 11
NOUT = 16

_NC_CACHE = None


def _ap(handle_ap, off, dims):
    return bass.AP(handle_ap.tensor, off, [list(d) for d in dims])


def _bcast_last(ap, n):
    return bass.AP(ap.tensor, ap.offset, [list(d) for d in ap.ap] + [[0, n]])


def _build_nc():
    nc = bacc.Bacc("TRN2", target_bir_lowering=False, debug=False)
    fcls_t = nc.dram_tensor("FCLS", [NCELL_PAD * NUM_CLASSES], F32,
                            kind="ExternalInput")
    fbox_t = nc.dram_tensor("FBOX", [NCELL * 5], F32, kind="ExternalInput")
    obj_t = nc.dram_tensor("OBJ", [NOBJ], F32, kind="ExternalInput")
    wc_t = nc.dram_tensor("WC", [NCELL_PAD], F32, kind="ExternalInput")
    gi_t = nc.dram_tensor("GI", [128, 8], I32, kind="ExternalInput")
    mt_t = nc.dram_tensor("MT", [128, NMETA], F32, kind="ExternalInput")
    out_t = nc.dram_tensor("OUT", [128, NOUT], F32, kind="ExternalOutput")

    # softplus(x) = ln(exp(x) + 1): Exp and Ln live in the same ACT table
    # (natural_log_exp_and_others), so no table swaps.  Safe for randn-scale
    # logits (exp overflows only past ~88).  Sigmoid = 1/(1+exp(-x)) via DVE
    # reciprocal for the same reason.
    EXP = mybir.ActivationFunctionType.Exp
    LN = mybir.ActivationFunctionType.Ln
    A = mybir.AluOpType
    with TileContext(nc) as tc:
        with tc.tile_pool(name="persist", bufs=1) as pp, \
             tc.tile_pool(name="dense", bufs=3) as dp, \
             tc.tile_pool(name="wrow", bufs=3) as wp:
            part = pp.tile([128, NOUT], F32)
            nc.vector.memset(part[:], 0.0)

            # ---- sparse per-target part ----
            gi = pp.tile([128, 8], I32)
            mt = pp.tile([128, NMETA], F32)
            va = pp.tile([128, 20], F32)   # gathered [128, 4, 5] rows
            vb = pp.tile([128, 4], F32)    # gathered class logits
            l1 = pp.tile([128, 4], F32)
            sc = pp.tile([128, 4], F32)
            nc.sync.dma_start(out=gi[:], in_=gi_t.ap())
            nc.sync.dma_start(out=mt[:], in_=mt_t.ap())
            # indirect DMA gathers one contiguous row per partition, so
            # 512 targets = 4 calls; target t lives at (p, j) = (t%128, t//128)
            for j in range(4):
                nc.gpsimd.indirect_dma_start(
                    out=va[:, 5 * j:5 * j + 5], out_offset=None,
                    in_=_ap(fbox_t.ap(), 0, [[1, NCELL * 5], [1, 1]]),
                    in_offset=bass.IndirectOffsetOnAxis(ap=gi[:, j:j + 1],
                                                        axis=0))
                nc.gpsimd.indirect_dma_start(
                    out=vb[:, j:j + 1], out_offset=None,
                    in_=_ap(fcls_t.ap(), 0,
                            [[1, NCELL_PAD * NUM_CLASSES], [1, 1]]),
                    in_offset=bass.IndirectOffsetOnAxis(ap=gi[:, 4 + j:5 + j],
                                                        axis=0))

            va3 = va[:].rearrange("p (j c) -> p j c", c=5)
            mt3 = lambda lo, w: mt[:, lo:lo + 4 * w].rearrange(
                "p (j c) -> p j c", c=w)
            # decode: ch0,1 -> sigmoid + gx,gy ; ch2,3 -> exp(min(x,4))
            nc.scalar.activation(va3[:, :, 0:2], va3[:, :, 0:2], EXP,
                                 scale=-1.0)
            nc.vector.tensor_scalar_add(va3[:, :, 0:2], va3[:, :, 0:2], 1.0)
            nc.vector.reciprocal(va3[:, :, 0:2], va3[:, :, 0:2])
            nc.vector.tensor_scalar_min(va3[:, :, 2:4], va3[:, :, 2:4], 4.0)
            nc.scalar.activation(va3[:, :, 2:4], va3[:, :, 2:4], EXP)
            nc.vector.tensor_add(va3[:, :, 0:2], va3[:, :, 0:2], mt3(MC_ADD, 2))
            nc.vector.tensor_mul(va3[:, :, 0:4], va3[:, :, 0:4], mt3(MC_MUL, 4))
            nc.vector.tensor_sub(va3[:, :, 0:4], va3[:, :, 0:4], mt3(MC_SUB, 4))
            nc.scalar.activation(va3[:, :, 0:4], va3[:, :, 0:4],
                                 mybir.ActivationFunctionType.Abs)
            # (tensor_tensor_reduce is broken on this HW build; use
            # explicit multiply + tensor_reduce instead)
            AX = mybir.AxisListType
            nc.vector.tensor_add(l1[:], va3[:, :, 0], va3[:, :, 1])
            nc.vector.tensor_add(l1[:], l1[:], va3[:, :, 2])
            nc.vector.tensor_add(l1[:], l1[:], va3[:, :, 3])
            nc.vector.tensor_mul(l1[:], l1[:], mt[:, MC_SWM:MC_SWM + 4])
            nc.vector.reduce_sum(part[:, OC_BOX:OC_BOX + 1], l1[:], axis=AX.X)
            for s in range(3):
                nc.vector.tensor_mul(sc[:], va3[:, :, 4],
                                     mt[:, MC_D0 + 4 * s:MC_D0 + 4 * s + 4])
                nc.vector.reduce_sum(part[:, OC_POS + s:OC_POS + s + 1],
                                     sc[:], axis=AX.X)
            nc.vector.tensor_mul(vb[:], vb[:], mt[:, MC_VC:MC_VC + 4])
            nc.vector.reduce_sum(part[:, OC_CORR:OC_CORR + 1], vb[:],
                                 axis=AX.X)

            # ---- dense objectness stream (per scale) ----
            ob = pp.tile([128, sum(OBJ_COLS)], F32)
            ocol = 0
            ooff = 0
            for s in range(3):
                w = OBJ_COLS[s]
                nc.sync.dma_start(out=ob[:, ocol:ocol + w],
                                  in_=_ap(obj_t.ap(), ooff, [[w, 128], [1, w]]))
                nc.scalar.activation(ob[:, ocol:ocol + w],
                                     ob[:, ocol:ocol + w], EXP)
                nc.scalar.activation(
                    ob[:, ocol:ocol + w], ob[:, ocol:ocol + w], LN, bias=1.0,
                    accum_out=part[:, OC_OBJ + s:OC_OBJ + s + 1])
                ocol += w
                ooff += 128 * w

            # ---- dense class stream ----
            # per chunk: softplus, reduce over the 80 classes, weight by
            # the per-cell target count, reduce over cells
            for c in range(NCHUNK):
                dt = dp.tile([128, FREE], F32, tag="dt")
                wcl = wp.tile([128, KCELL], F32, tag="wcl")
                g = wp.tile([128, KCELL], F32, tag="g")
                nc.sync.dma_start(
                    out=dt[:],
                    in_=_ap(fcls_t.ap(), c * CELLS_PC * NUM_CLASSES,
                            [[FREE, 128], [1, FREE]]))
                nc.sync.dma_start(
                    out=wcl[:],
                    in_=_ap(wc_t.ap(), c * CELLS_PC, [[KCELL, 128], [1, KCELL]]))
                nc.scalar.activation(dt[:], dt[:], EXP)
                nc.scalar.activation(dt[:], dt[:], LN, bias=1.0)
                dt3 = dt[:].rearrange("p (a b) -> p a b", b=NUM_CLASSES)
                nc.vector.reduce_sum(g[:], dt3, axis=mybir.AxisListType.X)
                nc.vector.tensor_mul(g[:], g[:], wcl[:])
                nc.vector.reduce_sum(part[:, OC_WSP + c:OC_WSP + c + 1],
                                     g[:], axis=mybir.AxisListType.X)

            nc.sync.dma_start(out=out_t.ap(), in_=part[:])
    nc.compile()
    return nc


def get_nc():
    global _NC_CACHE
    if _NC_CACHE is None:
        _NC_CACHE = _build_nc()
    return _NC_CACHE


def prepare_in_maps(pred0, pred1, pred2, targets):
    """Host-side sharding + layout/index preprocessing (numpy only)."""
    preds = (np.asarray(pred0, dtype=np.float32),
             np.asarray(pred1, dtype=np.float32),
             np.asarray(pred2, dtype=np.float32))
    t = np.asarray(targets, dtype=np.float32)
    n = t.shape[0]
    b = t[:, 0].astype(np.int32)
    cls = t[:, 1].astype(np.int32)
    cx, cy, bw, bh = t[:, 2], t[:, 3], t[:, 4], t[:, 5]

    area = np.maximum(bw * bh, np.float32(1e-6))
    s_idx = np.where(area <= 0.01, 0,
                     np.where(area <= 0.03, 1, 2)).astype(np.int32)
    sw = np.float32(1.0) + STAL_GAMMA * (np.float32(1.0) - np.sqrt(area))

    ws = np.array(WS, np.int32)[s_idx]
    wf = ws.astype(np.float32)
    gx = np.clip((cx * wf).astype(np.int32), 0, ws - 1)
    gy = np.clip((cy * wf).astype(np.int32), 0, ws - 1)
    hw = np.array(HW, np.int64)[s_idx]

    b_cl = np.clip(b, 0, BATCH - 1)
    core = b_cl // BPC
    bl = (b_cl % BPC).astype(np.int64)
    cell = (np.array(COFF, np.int64)[s_idx] + bl * hw
            + (gy.astype(np.int64) * ws + gx))

    valid_cls = ((cls >= 0) & (cls < NUM_CLASSES)).astype(np.float32)
    cls_c = np.clip(cls, 0, NUM_CLASSES - 1).astype(np.int64)

    # obj dedup: one representative target per (scale, batch, gy, gx) cell
    key = ((s_idx.astype(np.int64) * BATCH + b_cl) * 128 + gy) * 128 + gx
    dflag = np.zeros(n, np.float32)
    _, first = np.unique(key, return_index=True)
    dflag[first] = 1.0

    in_maps = []
    for c in range(NCORES):
        sel = np.nonzero(core == c)[0]
        if len(sel) > TPAD:
            sel = sel[:TPAD]  # graceful degradation; never expected
        m = len(sel)
        csel = cell[sel]

        # target t maps to (partition, group) = (t % 128, t // 128)
        gi = np.zeros((128, 8), np.int32)  # cols 0..3 = gather A, 4..7 = B
        ga = np.zeros(TPAD, np.int64)
        gb = np.zeros(TPAD, np.int64)
        ga[:m] = csel * 5
        gb[:m] = csel * NUM_CLASSES + cls_c[sel]
        gi[:, 0:4] = ga.astype(np.int32).reshape(4, 128).T
        gi[:, 4:8] = gb.astype(np.int32).reshape(4, 128).T

        mt = np.zeros((128, NMETA), np.float32)

        def put(col, vals):
            buf = np.zeros(TPAD, np.float32)
            buf[:m] = vals
            mt[:, col:col + 4] = buf.reshape(4, 128).T

        def put_il(col, width, *vals):  # channel-interleaved group
            buf = np.zeros((TPAD, width), np.float32)
            for i, v in enumerate(vals):
                buf[:m, i] = v
            mt[:, col:col + 4 * width] = buf.reshape(
                4, 128, width).transpose(1, 0, 2).reshape(128, 4 * width)

        invw = np.float32(1.0) / wf[sel]
        put_il(MC_ADD, 2, gx[sel].astype(np.float32),
               gy[sel].astype(np.float32))
        put_il(MC_MUL, 4, invw, invw, invw, invw)
        put_il(MC_SUB, 4, cx[sel], cy[sel], bw[sel], bh[sel])
        put(MC_SWM, sw[sel] * np.float32(0.25))
        for s in range(3):
            put(MC_D0 + 4 * s, dflag[sel] * (s_idx[sel] == s))
        put(MC_VC, valid_cls[sel])

        wc = np.zeros(NCELL_PAD, np.float32)
        np.add.at(wc, csel, np.float32(1.0))

        lo, hi = c * BPC, (c + 1) * BPC
        fcls = np.empty((NCELL_PAD, NUM_CLASSES), np.float32)
        fbox = np.empty((NCELL, 5), np.float32)
        obj = np.full(NOBJ, np.float32(-100.0), np.float32)
        off = 0
        ooff = 0
        for s, p in enumerate(preds):
            nc_s = BPC * HW[s]
            blk = p[lo:hi].reshape(BPC, CH, HW[s])
            fcls[off:off + nc_s] = np.moveaxis(
                blk[:, 5:], 1, 2).reshape(nc_s, NUM_CLASSES)
            fbox[off:off + nc_s] = np.moveaxis(
                blk[:, 0:5], 1, 2).reshape(nc_s, 5)
            obj[ooff:ooff + nc_s] = blk[:, 4].reshape(-1)
            off += nc_s
            ooff += nc_s
        fcls[NCELL:] = 0.0

        in_maps.append({
            "FCLS": fcls.reshape(-1),
            "FBOX": fbox.reshape(-1),
            "OBJ": obj,
            "WC": wc,
            "GI": gi,
            "MT": mt,
        })
    return in_maps, n


def finalize(results, n):
    """Combine per-core [128, NOUT] partial tiles into the 4 losses."""
    ps = np.stack([np.asarray(r["OUT"], np.float64) for r in results])
    cls_dense = ps[:, :, OC_WSP:OC_WSP + NCHUNK].sum()
    obj_sp = [ps[:, :, OC_OBJ + s].sum() for s in range(3)]
    box = ps[:, :, OC_BOX].sum()
    pos = [ps[:, :, OC_POS + s].sum() for s in range(3)]
    corr = ps[:, :, OC_CORR].sum()

    norm = max(1, n)
    box_loss = box / norm
    cls_loss = (cls_dense - corr) / (NUM_CLASSES * norm)
    obj_loss = sum((obj_sp[s] - pos[s]) / (BATCH * HW[s]) for s in range(3))
    total = box_loss + obj_loss + cls_loss
    return np.array([total, box_loss, obj_loss, cls_loss], np.float32)


def run_on_hw(in_maps, trace=False):
    nc = get_nc()
    return bass_utils.run_bass_kernel_spmd(
        nc, in_maps, core_ids=list(range(NCORES)), trace=trace)


def kernel(pred0, pred1, pred2, targets, **_unused):
    in_maps, n = prepare_in_maps(pred0, pred1, pred2, targets)
    res = run_on_hw(in_maps)
    return finalize(res.results, n)


# revision 17
# speedup vs baseline: 2.7055x; 2.7055x over previous
"""Trainium2 Bass kernel for a YOLO-style detection loss.

Sharding: data-parallel over batch — 8 NeuronCores, 4 batches/core.
Per-core partial sums land in a [128, 16] tile; the host sums the
relevant slices of the 8 tiles and assembles the 4 scalar losses
(this host gather replaces the all-reduce of 4 scalars).

Key observation: the loss only touches pred densely through the
objectness channel (BCE vs 0 over every cell).  The class BCE term
decomposes as

    cls = [ sum_t sum_k softplus(x[cell_t, k]) - sum_t x[cell_t, cls_t]
          ] / (80 * n)

i.e. it needs the 80 class logits only at the ~2048 assigned cells, and
the box term needs channels 0..3 at those cells.  So the device work is

1. OBJ stream: softplus-sum of channel 4 over all cells, per scale
   (ACT with fused per-partition accumulate); positive-cell correction
   (BCE(x,1)-BCE(x,0) = -x) comes from the gathered rows.
2. One 85-float channels-last row gather per target (indirect DMA,
   contiguous rows from a host-transposed [cells, 85] copy), plus a
   1-float gather of the target-class logit, then a few dozen small
   DVE/ACT ops for the box decode / l1 / class softplus sums.

softplus(x) = ln(exp(x) + 1): Exp and Ln share one ACT table
(natural_log_exp_and_others) so there are no table swaps; safe for
randn-scale logits.  Sigmoid = 1/(1+exp(-x)) via DVE reciprocal.
(tensor_tensor_reduce is broken on this HW build — multiply +
tensor_reduce is used instead.)
"""

import numpy as np

from concourse import bass, bacc, mybir
from concourse import bass_utils
from concourse.tile import TileContext

F32 = mybir.dt.float32
I32 = mybir.dt.int32

NUM_CLASSES = 80
STAL_GAMMA = np.float32(2.0)
BATCH = 32
NCORES = 8
BPC = BATCH // NCORES          # batches per core
CH = 5 + NUM_CLASSES
HW = (80 * 80, 40 * 40, 20 * 20)
WS = (80, 40, 20)
NCELL = BPC * (HW[0] + HW[1] + HW[2])       # 33600 cells per core
COFF = (0, BPC * HW[0], BPC * (HW[0] + HW[1]))  # per-scale cell offsets
# OBJ stream: per-scale tiles, scale 2 padded to a multiple of 128
OBJ_COLS = (HW[0] * BPC // 128, HW[1] * BPC // 128, 1664 // 128)  # 200,50,13
NOBJ = HW[0] * BPC + HW[1] * BPC + 1664     # 33664 (64 pad cells of -100)
TPAD = 512                                  # per-core target capacity
# meta column layout (groups of 4 target-columns, channel-interleaved)
MC_ADD = 0      # (gx, gy) per target            cols 0..7
MC_MUL = 8      # 1/w x4                         cols 8..23
MC_SUB = 24     # (cx, cy, bw, bh)               cols 24..39
MC_SWM = 40     # small_weight/4 (0 for pads)    cols 40..43
MC_D0 = 44      # obj dedup flag per scale       cols 44..55
MC_VC = 56      # valid-class flag               cols 56..59
MC_VLD = 60     # real-target flag               cols 60..63
NMETA = 64
# output partial tile column layout
OC_WSP = 0      # class softplus-sum term
OC_OBJ = 1      # 3 cols: per-scale objectness softplus sums
OC_BOX = 4
OC_POS = 5      # 3 cols
OC_CORR = 8
NOUT = 16

_NC_CACHE = None


def _ap(handle_ap, off, dims):
    return bass.AP(handle_ap.tensor, off, [list(d) for d in dims])


def _build_nc():
    nc = bacc.Bacc("TRN2", target_bir_lowering=False, debug=False)
    fall_t = nc.dram_tensor("FALL", [NCELL * CH], F32, kind="ExternalInput")
    obj_t = nc.dram_tensor("OBJ", [NOBJ], F32, kind="ExternalInput")
    gi_t = nc.dram_tensor("GI", [128, 8], I32, kind="ExternalInput")
    mt_t = nc.dram_tensor("MT", [128, NMETA], F32, kind="ExternalInput")
    out_t = nc.dram_tensor("OUT", [128, NOUT], F32, kind="ExternalOutput")

    EXP = mybir.ActivationFunctionType.Exp
    LN = mybir.ActivationFunctionType.Ln
    ABS = mybir.ActivationFunctionType.Abs
    A = mybir.AluOpType
    AX = mybir.AxisListType
    with TileContext(nc) as tc:
        with tc.tile_pool(name="persist", bufs=1) as pp:
            part = pp.tile([128, NOUT], F32)
            nc.vector.memset(part[:], 0.0)

            gi = pp.tile([128, 8], I32)
            mt = pp.tile([128, NMETA], F32)
            va = pp.tile([128, 4 * CH], F32)   # per-target 85-float rows
            vb = pp.tile([128, 4], F32)        # per-target class logits
            l1 = pp.tile([128, 4], F32)
            sc = pp.tile([128, 4], F32)
            g4 = pp.tile([128, 4], F32)
            nc.sync.dma_start(out=gi[:], in_=gi_t.ap())
            nc.sync.dma_start(out=mt[:], in_=mt_t.ap())
            # one row gather + one element gather per target; 128 rows
            # per call (one per partition), target t at (p, j) =
            # (t % 128, t // 128)
            for j in range(4):
                nc.gpsimd.indirect_dma_start(
                    out=va[:, CH * j:CH * j + CH], out_offset=None,
                    in_=_ap(fall_t.ap(), 0, [[1, NCELL * CH], [1, 1]]),
                    in_offset=bass.IndirectOffsetOnAxis(ap=gi[:, j:j + 1],
                                                        axis=0))
                nc.gpsimd.indirect_dma_start(
                    out=vb[:, j:j + 1], out_offset=None,
                    in_=_ap(fall_t.ap(), 0, [[1, NCELL * CH], [1, 1]]),
                    in_offset=bass.IndirectOffsetOnAxis(ap=gi[:, 4 + j:5 + j],
                                                        axis=0))

            # ---- dense objectness stream (per scale) ----
            ob = pp.tile([128, sum(OBJ_COLS)], F32)
            ocol = 0
            ooff = 0
            for s in range(3):
                w = OBJ_COLS[s]
                nc.sync.dma_start(out=ob[:, ocol:ocol + w],
                                  in_=_ap(obj_t.ap(), ooff, [[w, 128], [1, w]]))
                nc.scalar.activation(ob[:, ocol:ocol + w],
                                     ob[:, ocol:ocol + w], EXP)
                nc.scalar.activation(
                    ob[:, ocol:ocol + w], ob[:, ocol:ocol + w], LN, bias=1.0,
                    accum_out=part[:, OC_OBJ + s:OC_OBJ + s + 1])
                ocol += w
                ooff += 128 * w

            # ---- per-target math ----
            va3 = va[:].rearrange("p (j c) -> p j c", c=CH)
            mt3 = lambda lo, w: mt[:, lo:lo + 4 * w].rearrange(
                "p (j c) -> p j c", c=w)
            # box decode: ch0,1 -> sigmoid + gx,gy ; ch2,3 -> exp(min(x,4))
            nc.scalar.activation(va3[:, :, 0:2], va3[:, :, 0:2], EXP,
                                 scale=-1.0)
            nc.vector.tensor_scalar_add(va3[:, :, 0:2], va3[:, :, 0:2], 1.0)
            nc.vector.reciprocal(va3[:, :, 0:2], va3[:, :, 0:2])
            nc.vector.tensor_scalar_min(va3[:, :, 2:4], va3[:, :, 2:4], 4.0)
            nc.scalar.activation(va3[:, :, 2:4], va3[:, :, 2:4], EXP)
            nc.vector.tensor_add(va3[:, :, 0:2], va3[:, :, 0:2], mt3(MC_ADD, 2))
            nc.vector.tensor_mul(va3[:, :, 0:4], va3[:, :, 0:4], mt3(MC_MUL, 4))
            nc.vector.tensor_sub(va3[:, :, 0:4], va3[:, :, 0:4], mt3(MC_SUB, 4))
            nc.scalar.activation(va3[:, :, 0:4], va3[:, :, 0:4], ABS)
            nc.vector.tensor_add(l1[:], va3[:, :, 0], va3[:, :, 1])
            nc.vector.tensor_add(l1[:], l1[:], va3[:, :, 2])
            nc.vector.tensor_add(l1[:], l1[:], va3[:, :, 3])
            nc.vector.tensor_mul(l1[:], l1[:], mt[:, MC_SWM:MC_SWM + 4])
            nc.vector.reduce_sum(part[:, OC_BOX:OC_BOX + 1], l1[:], axis=AX.X)
            # objectness positive-cell correction (raw channel 4)
            for s in range(3):
                nc.vector.tensor_mul(sc[:], va3[:, :, 4],
                                     mt[:, MC_D0 + 4 * s:MC_D0 + 4 * s + 4])
                nc.vector.reduce_sum(part[:, OC_POS + s:OC_POS + s + 1],
                                     sc[:], axis=AX.X)
            # class softplus sum over the 80 logits of each target's cell
            nc.scalar.activation(va3[:, :, 5:CH], va3[:, :, 5:CH], EXP)
            nc.scalar.activation(va3[:, :, 5:CH], va3[:, :, 5:CH], LN,
                                 bias=1.0)
            nc.vector.reduce_sum(g4[:], va3[:, :, 5:CH], axis=AX.X)
            nc.vector.tensor_mul(g4[:], g4[:], mt[:, MC_VLD:MC_VLD + 4])
            nc.vector.reduce_sum(part[:, OC_WSP:OC_WSP + 1], g4[:], axis=AX.X)
            # per-target class-logit correction
            nc.vector.tensor_mul(vb[:], vb[:], mt[:, MC_VC:MC_VC + 4])
            nc.vector.reduce_sum(part[:, OC_CORR:OC_CORR + 1], vb[:],
                                 axis=AX.X)

            nc.sync.dma_start(out=out_t.ap(), in_=part[:])
    nc.compile()
    return nc


def get_nc():
    global _NC_CACHE
    if _NC_CACHE is None:
        _NC_CACHE = _build_nc()
    return _NC_CACHE


def prepare_in_maps(pred0, pred1, pred2, targets):
    """Host-side sharding + layout/index preprocessing (numpy only)."""
    preds = (np.asarray(pred0, dtype=np.float32),
             np.asarray(pred1, dtype=np.float32),
             np.asarray(pred2, dtype=np.float32))
    t = np.asarray(targets, dtype=np.float32)
    n = t.shape[0]
    b = t[:, 0].astype(np.int32)
    cls = t[:, 1].astype(np.int32)
    cx, cy, bw, bh = t[:, 2], t[:, 3], t[:, 4], t[:, 5]

    area = np.maximum(bw * bh, np.float32(1e-6))
    s_idx = np.where(area <= 0.01, 0,
                     np.where(area <= 0.03, 1, 2)).astype(np.int32)
    sw = np.float32(1.0) + STAL_GAMMA * (np.float32(1.0) - np.sqrt(area))

    ws = np.array(WS, np.int32)[s_idx]
    wf = ws.astype(np.float32)
    gx = np.clip((cx * wf).astype(np.int32), 0, ws - 1)
    gy = np.clip((cy * wf).astype(np.int32), 0, ws - 1)
    hw = np.array(HW, np.int64)[s_idx]

    b_cl = np.clip(b, 0, BATCH - 1)
    core = b_cl // BPC
    bl = (b_cl % BPC).astype(np.int64)
    cell = (np.array(COFF, np.int64)[s_idx] + bl * hw
            + (gy.astype(np.int64) * ws + gx))

    valid_cls = ((cls >= 0) & (cls < NUM_CLASSES)).astype(np.float32)
    cls_c = np.clip(cls, 0, NUM_CLASSES - 1).astype(np.int64)

    # obj dedup: one representative target per (scale, batch, gy, gx) cell
    key = ((s_idx.astype(np.int64) * BATCH + b_cl) * 128 + gy) * 128 + gx
    dflag = np.zeros(n, np.float32)
    _, first = np.unique(key, return_index=True)
    dflag[first] = 1.0

    in_maps = []
    for c in range(NCORES):
        sel = np.nonzero(core == c)[0]
        if len(sel) > TPAD:
            sel = sel[:TPAD]  # graceful degradation; never expected
        m = len(sel)
        csel = cell[sel]

        # target t maps to (partition, group) = (t % 128, t // 128)
        gi = np.zeros((128, 8), np.int32)  # cols 0..3 = rows, 4..7 = logits
        ga = np.zeros(TPAD, np.int64)
        gb = np.zeros(TPAD, np.int64)
        ga[:m] = csel * CH
        gb[:m] = csel * CH + 5 + cls_c[sel]
        gi[:, 0:4] = ga.astype(np.int32).reshape(4, 128).T
        gi[:, 4:8] = gb.astype(np.int32).reshape(4, 128).T

        mt = np.zeros((128, NMETA), np.float32)

        def put(col, vals):
            buf = np.zeros(TPAD, np.float32)
            buf[:m] = vals
            mt[:, col:col + 4] = buf.reshape(4, 128).T

        def put_il(col, width, *vals):  # channel-interleaved group
            buf = np.zeros((TPAD, width), np.float32)
            for i, v in enumerate(vals):
                buf[:m, i] = v
            mt[:, col:col + 4 * width] = buf.reshape(
                4, 128, width).transpose(1, 0, 2).reshape(128, 4 * width)

        invw = np.float32(1.0) / wf[sel]
        put_il(MC_ADD, 2, gx[sel].astype(np.float32),
               gy[sel].astype(np.float32))
        put_il(MC_MUL, 4, invw, invw, invw, invw)
        put_il(MC_SUB, 4, cx[sel], cy[sel], bw[sel], bh[sel])
        put(MC_SWM, sw[sel] * np.float32(0.25))
        for s in range(3):
            put(MC_D0 + 4 * s, dflag[sel] * (s_idx[sel] == s))
        put(MC_VC, valid_cls[sel])
        put(MC_VLD, np.float32(1.0))

        lo, hi = c * BPC, (c + 1) * BPC
        fall = np.empty((NCELL, CH), np.float32)
        obj = np.full(NOBJ, np.float32(-100.0), np.float32)
        off = 0
        ooff = 0
        for s, p in enumerate(preds):
            nc_s = BPC * HW[s]
            blk = p[lo:hi].reshape(BPC, CH, HW[s])
            fall[off:off + nc_s] = np.moveaxis(blk, 1, 2).reshape(nc_s, CH)
            obj[ooff:ooff + nc_s] = blk[:, 4].reshape(-1)
            off += nc_s
            ooff += nc_s

        in_maps.append({
            "FALL": fall.reshape(-1),
            "OBJ": obj,
            "GI": gi,
            "MT": mt,
        })
    return in_maps, n


def finalize(results, n):
    """Combine per-core [128, NOUT] partial tiles into the 4 losses."""
    ps = np.stack([np.asarray(r["OUT"], np.float64) for r in results])
    cls_sp = ps[:, :, OC_WSP].sum()
    obj_sp = [ps[:, :, OC_OBJ + s].sum() for s in range(3)]
    box = ps[:, :, OC_BOX].sum()
    pos = [ps[:, :, OC_POS + s].sum() for s in range(3)]
    corr = ps[:, :, OC_CORR].sum()

    norm = max(1, n)
    box_loss = box / norm
    cls_loss = (cls_sp - corr) / (NUM_CLASSES * norm)
    obj_loss = sum((obj_sp[s] - pos[s]) / (BATCH * HW[s]) for s in range(3))
    total = box_loss + obj_loss + cls_loss
    return np.array([total, box_loss, obj_loss, cls_loss], np.float32)


def run_on_hw(in_maps, trace=False):
    nc = get_nc()
    return bass_utils.run_bass_kernel_spmd(
        nc, in_maps, core_ids=list(range(NCORES)), trace=trace)


def kernel(pred0, pred1, pred2, targets, **_unused):
    in_maps, n = prepare_in_maps(pred0, pred1, pred2, targets)
    res = run_on_hw(in_maps)
    return finalize(res.results, n)


# revision 18
# speedup vs baseline: 3.5899x; 1.3269x over previous
"""Trainium2 Bass kernel for a YOLO-style detection loss.

Sharding: data-parallel over batch — 8 NeuronCores, 4 batches/core.
Per-core partial sums land in a [128, 16] tile; the host sums the
relevant slices of the 8 tiles and assembles the 4 scalar losses
(this host gather replaces the all-reduce of 4 scalars).

Key observation: the loss only touches pred densely through the
objectness channel (BCE vs 0 over every cell).  The class BCE term
needs the 80 class logits only at the assigned cells, and the box term
needs channels 0..3 there.  Device work:

1. OBJ stream: softplus over channel 4 of every cell (one [128, 263]
   tile), per-scale sums via DVE column reduces; the positive-cell
   correction (BCE(x,1)-BCE(x,0) = -x) comes from the gathered rows.
2. One 85-float channels-last row gather per target (indirect DMA;
   contiguous rows of a host-transposed [cells, 85] copy; 128 rows per
   call, 3 calls for up to 384 targets/core), then ~25 small DVE/ACT
   ops: box decode + l1, per-scale positive sums, class softplus sums,
   and the target-class logit correction as a one-hot dot product.

softplus(x) = ln(exp(x) + 1); Exp/Ln/Abs are pinned to the single ACT
table that holds all three (natural_log_exp_and_others) to avoid
per-instruction table reloads.  Sigmoid = 1/(1+exp(-x)) via DVE
reciprocal.  tensor_tensor_reduce is broken on this HW build, so
reductions use multiply + tensor_reduce.
"""

import numpy as np

from concourse import bass, bacc, mybir
from concourse import bass_utils
from concourse.tile import TileContext

F32 = mybir.dt.float32
I32 = mybir.dt.int32

NUM_CLASSES = 80
STAL_GAMMA = np.float32(2.0)
BATCH = 32
NCORES = 8
BPC = BATCH // NCORES          # batches per core
CH = 5 + NUM_CLASSES
HW = (80 * 80, 40 * 40, 20 * 20)
WS = (80, 40, 20)
NCELL = BPC * (HW[0] + HW[1] + HW[2])       # 33600 cells per core
COFF = (0, BPC * HW[0], BPC * (HW[0] + HW[1]))  # per-scale cell offsets
# OBJ stream: per-scale column blocks, scale 2 padded to 128*13
OBJ_COLS = (HW[0] * BPC // 128, HW[1] * BPC // 128, 1664 // 128)  # 200,50,13
NOBJ = HW[0] * BPC + HW[1] * BPC + 1664     # 33664 (64 pad cells of -100)
GROUPS = 3                                  # gather calls (128 targets each)
TPAD = 128 * GROUPS                         # 384; mean load is ~256/core
# meta column layout (GROUPS target-columns per quantity, interleaved)
MC_ADD = 0                                  # (gx, gy)          6 cols
MC_MUL = 6                                  # 1/w x4           12 cols
MC_SUB = 18                                 # (cx, cy, bw, bh) 12 cols
MC_SWM = 30                                 # small_weight/4    3 cols
MC_D0 = 33                                  # obj dedup flags   9 cols
MC_VLD = 42                                 # real-target flag  3 cols
MC_OH = 48                                  # class one-hot   240 cols
NMETA = MC_OH + GROUPS * NUM_CLASSES        # 288
# output partial tile column layout
OC_WSP = 0      # class softplus-sum term
OC_OBJ = 1      # 3 cols: per-scale objectness softplus sums
OC_BOX = 4
OC_POS = 5      # 3 cols
OC_CORR = 8
NOUT = 16

_NC_CACHE = None


def _ap(handle_ap, off, dims):
    return bass.AP(handle_ap.tensor, off, [list(d) for d in dims])


def _single_act_table(arch):
    """All of Exp/Ln/Abs live in natural_log_exp_and_others; hide them
    from the other tables so every activation uses one table (one load
    instead of a reload on each Exp<->Ln transition)."""
    tabs = _ORIG_TABLES(arch)
    need = {mybir.ActivationFunctionType.Exp,
            mybir.ActivationFunctionType.Ln,
            mybir.ActivationFunctionType.Abs}
    out = {}
    for name, fns in tabs.items():
        out[name] = fns if name == "natural_log_exp_and_others" \
            else fns - need
    return out


_ORIG_TABLES = bacc.get_activation_tables


def _build_nc():
    nc = bacc.Bacc("TRN2", target_bir_lowering=False, debug=False)
    fall_t = nc.dram_tensor("FALL", [NCELL * CH], F32, kind="ExternalInput")
    obj_t = nc.dram_tensor("OBJ", [NOBJ], F32, kind="ExternalInput")
    gi_t = nc.dram_tensor("GI", [128, GROUPS], I32, kind="ExternalInput")
    mt_t = nc.dram_tensor("MT", [128, NMETA], F32, kind="ExternalInput")
    out_t = nc.dram_tensor("OUT", [128, NOUT], F32, kind="ExternalOutput")

    EXP = mybir.ActivationFunctionType.Exp
    LN = mybir.ActivationFunctionType.Ln
    ABS = mybir.ActivationFunctionType.Abs
    AX = mybir.AxisListType
    NOB = sum(OBJ_COLS)
    with TileContext(nc) as tc:
        with tc.tile_pool(name="persist", bufs=1) as pp:
            part = pp.tile([128, NOUT], F32)
            nc.vector.memset(part[:], 0.0)

            gi = pp.tile([128, GROUPS], I32)
            mt = pp.tile([128, NMETA], F32)
            va = pp.tile([128, GROUPS * CH], F32)  # per-target 85-float rows
            vt = pp.tile([128, GROUPS * NUM_CLASSES], F32)
            l1 = pp.tile([128, GROUPS], F32)
            sc = pp.tile([128, GROUPS], F32)
            g3 = pp.tile([128, GROUPS], F32)
            ob = pp.tile([128, NOB], F32)
            nc.sync.dma_start(out=gi[:], in_=gi_t.ap())
            nc.sync.dma_start(out=mt[:], in_=mt_t.ap())
            # one 85-float row per target; 128 rows (one per partition)
            # per call; target t sits at (p, j) = (t % 128, t // 128)
            for j in range(GROUPS):
                nc.gpsimd.indirect_dma_start(
                    out=va[:, CH * j:CH * j + CH], out_offset=None,
                    in_=_ap(fall_t.ap(), 0, [[1, NCELL * CH], [1, 1]]),
                    in_offset=bass.IndirectOffsetOnAxis(ap=gi[:, j:j + 1],
                                                        axis=0))

            # ---- dense objectness stream ----
            ocol = 0
            ooff = 0
            for s in range(3):
                w = OBJ_COLS[s]
                nc.sync.dma_start(out=ob[:, ocol:ocol + w],
                                  in_=_ap(obj_t.ap(), ooff, [[w, 128], [1, w]]))
                ocol += w
                ooff += 128 * w
            nc.scalar.activation(ob[:], ob[:], EXP)
            nc.scalar.activation(ob[:], ob[:], LN, bias=1.0)
            ocol = 0
            for s in range(3):
                w = OBJ_COLS[s]
                nc.vector.reduce_sum(part[:, OC_OBJ + s:OC_OBJ + s + 1],
                                     ob[:, ocol:ocol + w], axis=AX.X)
                ocol += w

            # ---- per-target math ----
            va3 = va[:].rearrange("p (j c) -> p j c", c=CH)
            vt3 = vt[:].rearrange("p (j c) -> p j c", c=NUM_CLASSES)
            mt3 = lambda lo, w: mt[:, lo:lo + GROUPS * w].rearrange(
                "p (j c) -> p j c", c=w)
            # class-logit correction: one-hot dot with the raw logits
            nc.vector.tensor_mul(vt3, va3[:, :, 5:CH], mt3(MC_OH, NUM_CLASSES))
            nc.vector.reduce_sum(part[:, OC_CORR:OC_CORR + 1], vt[:],
                                 axis=AX.X)
            # box decode: ch0,1 -> sigmoid + gx,gy ; ch2,3 -> exp(min(x,4))
            nc.scalar.activation(va3[:, :, 0:2], va3[:, :, 0:2], EXP,
                                 scale=-1.0)
            nc.vector.tensor_scalar_add(va3[:, :, 0:2], va3[:, :, 0:2], 1.0)
            nc.vector.reciprocal(va3[:, :, 0:2], va3[:, :, 0:2])
            nc.vector.tensor_scalar_min(va3[:, :, 2:4], va3[:, :, 2:4], 4.0)
            nc.scalar.activation(va3[:, :, 2:4], va3[:, :, 2:4], EXP)
            nc.vector.tensor_add(va3[:, :, 0:2], va3[:, :, 0:2], mt3(MC_ADD, 2))
            nc.vector.tensor_mul(va3[:, :, 0:4], va3[:, :, 0:4], mt3(MC_MUL, 4))
            nc.vector.tensor_sub(va3[:, :, 0:4], va3[:, :, 0:4], mt3(MC_SUB, 4))
            nc.scalar.activation(va3[:, :, 0:4], va3[:, :, 0:4], ABS)
            nc.vector.tensor_add(l1[:], va3[:, :, 0], va3[:, :, 1])
            nc.vector.tensor_add(l1[:], l1[:], va3[:, :, 2])
            nc.vector.tensor_add(l1[:], l1[:], va3[:, :, 3])
            nc.vector.tensor_mul(l1[:], l1[:], mt[:, MC_SWM:MC_SWM + GROUPS])
            nc.vector.reduce_sum(part[:, OC_BOX:OC_BOX + 1], l1[:], axis=AX.X)
            # objectness positive-cell correction (raw channel 4)
            for s in range(3):
                nc.vector.tensor_mul(
                    sc[:], va3[:, :, 4],
                    mt[:, MC_D0 + GROUPS * s:MC_D0 + GROUPS * s + GROUPS])
                nc.vector.reduce_sum(part[:, OC_POS + s:OC_POS + s + 1],
                                     sc[:], axis=AX.X)
            # class softplus sum over the 80 logits of each target's cell
            nc.scalar.activation(va3[:, :, 5:CH], va3[:, :, 5:CH], EXP)
            nc.scalar.activation(va3[:, :, 5:CH], va3[:, :, 5:CH], LN,
                                 bias=1.0)
            nc.vector.reduce_sum(g3[:], va3[:, :, 5:CH], axis=AX.X)
            nc.vector.tensor_mul(g3[:], g3[:], mt[:, MC_VLD:MC_VLD + GROUPS])
            nc.vector.reduce_sum(part[:, OC_WSP:OC_WSP + 1], g3[:], axis=AX.X)

            nc.sync.dma_start(out=out_t.ap(), in_=part[:])
    bacc.get_activation_tables = _single_act_table
    try:
        nc.compile()
    finally:
        bacc.get_activation_tables = _ORIG_TABLES
    return nc


def get_nc():
    global _NC_CACHE
    if _NC_CACHE is None:
        _NC_CACHE = _build_nc()
    return _NC_CACHE


def prepare_in_maps(pred0, pred1, pred2, targets):
    """Host-side sharding + layout/index preprocessing (numpy only)."""
    preds = (np.asarray(pred0, dtype=np.float32),
             np.asarray(pred1, dtype=np.float32),
             np.asarray(pred2, dtype=np.float32))
    t = np.asarray(targets, dtype=np.float32)
    n = t.shape[0]
    b = t[:, 0].astype(np.int32)
    cls = t[:, 1].astype(np.int32)
    cx, cy, bw, bh = t[:, 2], t[:, 3], t[:, 4], t[:, 5]

    area = np.maximum(bw * bh, np.float32(1e-6))
    s_idx = np.where(area <= 0.01, 0,
                     np.where(area <= 0.03, 1, 2)).astype(np.int32)
    sw = np.float32(1.0) + STAL_GAMMA * (np.float32(1.0) - np.sqrt(area))

    ws = np.array(WS, np.int32)[s_idx]
    wf = ws.astype(np.float32)
    gx = np.clip((cx * wf).astype(np.int32), 0, ws - 1)
    gy = np.clip((cy * wf).astype(np.int32), 0, ws - 1)
    hw = np.array(HW, np.int64)[s_idx]

    b_cl = np.clip(b, 0, BATCH - 1)
    core = b_cl // BPC
    bl = (b_cl % BPC).astype(np.int64)
    cell = (np.array(COFF, np.int64)[s_idx] + bl * hw
            + (gy.astype(np.int64) * ws + gx))

    valid_cls = ((cls >= 0) & (cls < NUM_CLASSES)).astype(np.float32)
    cls_c = np.clip(cls, 0, NUM_CLASSES - 1)

    # obj dedup: one representative target per (scale, batch, gy, gx) cell
    key = ((s_idx.astype(np.int64) * BATCH + b_cl) * 128 + gy) * 128 + gx
    dflag = np.zeros(n, np.float32)
    _, first = np.unique(key, return_index=True)
    dflag[first] = 1.0

    in_maps = []
    for c in range(NCORES):
        sel = np.nonzero(core == c)[0]
        if len(sel) > TPAD:
            sel = sel[:TPAD]  # graceful degradation; never expected
        m = len(sel)
        csel = cell[sel]

        # target t maps to (partition, group) = (t % 128, t // 128)
        gi = np.zeros((128, GROUPS), np.int32)
        ga = np.zeros(TPAD, np.int64)
        ga[:m] = csel * CH
        gi[:] = ga.astype(np.int32).reshape(GROUPS, 128).T

        mt = np.zeros((128, NMETA), np.float32)

        def put(col, vals):
            buf = np.zeros(TPAD, np.float32)
            buf[:m] = vals
            mt[:, col:col + GROUPS] = buf.reshape(GROUPS, 128).T

        def put_il(col, width, *vals):  # channel-interleaved group
            buf = np.zeros((TPAD, width), np.float32)
            for i, v in enumerate(vals):
                buf[:m, i] = v
            mt[:, col:col + GROUPS * width] = buf.reshape(
                GROUPS, 128, width).transpose(1, 0, 2).reshape(
                128, GROUPS * width)

        invw = np.float32(1.0) / wf[sel]
        put_il(MC_ADD, 2, gx[sel].astype(np.float32),
               gy[sel].astype(np.float32))
        put_il(MC_MUL, 4, invw, invw, invw, invw)
        put_il(MC_SUB, 4, cx[sel], cy[sel], bw[sel], bh[sel])
        put(MC_SWM, sw[sel] * np.float32(0.25))
        for s in range(3):
            put(MC_D0 + GROUPS * s, dflag[sel] * (s_idx[sel] == s))
        put(MC_VLD, np.float32(1.0))
        oh = np.zeros((TPAD, NUM_CLASSES), np.float32)
        oh[np.arange(m), cls_c[sel]] = valid_cls[sel]
        mt[:, MC_OH:] = oh.reshape(GROUPS, 128, NUM_CLASSES).transpose(
            1, 0, 2).reshape(128, GROUPS * NUM_CLASSES)

        lo, hi = c * BPC, (c + 1) * BPC
        fall = np.empty((NCELL, CH), np.float32)
        obj = np.full(NOBJ, np.float32(-100.0), np.float32)
        off = 0
        ooff = 0
        for s, p in enumerate(preds):
            nc_s = BPC * HW[s]
            blk = p[lo:hi].reshape(BPC, CH, HW[s])
            fall[off:off + nc_s] = np.moveaxis(blk, 1, 2).reshape(nc_s, CH)
            obj[ooff:ooff + nc_s] = blk[:, 4].reshape(-1)
            off += nc_s
            ooff += nc_s

        in_maps.append({
            "FALL": fall.reshape(-1),
            "OBJ": obj,
            "GI": gi,
            "MT": mt,
        })
    return in_maps, n


def finalize(results, n):
    """Combine per-core [128, NOUT] partial tiles into the 4 losses."""
    ps = np.stack([np.asarray(r["OUT"], np.float64) for r in results])
    cls_sp = ps[:, :, OC_WSP].sum()
    obj_sp = [ps[:, :, OC_OBJ + s].sum() for s in range(3)]
    box = ps[:, :, OC_BOX].sum()
    pos = [ps[:, :, OC_POS + s].sum() for s in range(3)]
    corr = ps[:, :, OC_CORR].sum()

    norm = max(1, n)
    box_loss = box / norm
    cls_loss = (cls_sp - corr) / (NUM_CLASSES * norm)
    obj_loss = sum((obj_sp[s] - pos[s]) / (BATCH * HW[s]) for s in range(3))
    total = box_loss + obj_loss + cls_loss
    return np.array([total, box_loss, obj_loss, cls_loss], np.float32)


def run_on_hw(in_maps, trace=False):
    nc = get_nc()
    return bass_utils.run_bass_kernel_spmd(
        nc, in_maps, core_ids=list(range(NCORES)), trace=trace)


def kernel(pred0, pred1, pred2, targets, **_unused):
    in_maps, n = prepare_in_maps(pred0, pred1, pred2, targets)
    res = run_on_hw(in_maps)
    return finalize(res.results, n)
